# revision 22
# baseline (speedup 1.0000x reference)
"""Bahdanau attention kernel for Trainium2, SPMD across 8 NeuronCores.

Full inputs in, full outputs out. Sharding: data-parallel over batch B=64
(8 batches per core); W1/W2/V replicated.

Per-core math (B_loc=8, HW=1024 rows/batch, C=A=512):
  f_projT = (X @ W1)^T computed as W1_tile^T-stationary matmuls over
  X^T tiles obtained via X-bar DMA transpose of the bf16-cast features.
  tanh fused with the per-batch h_proj bias on the scalar engine.
  score = V . tanh(...) via matmuls whose stationary operand is V placed
  in column b (batch-selector), so scores land as [8, 1024] with batch on
  the partition axis. Softmax without max-subtraction (scores are O(1)):
  w = mask * exp(score + bV); alpha = w / sum(w); context = (w @ X) / sum(w).
"""

import numpy as np

import concourse.bass as bass
import concourse.mybir as mybir
import concourse.tile as tile
from concourse import bacc
from concourse.bass import ts, ds
from concourse.bass_utils import run_bass_kernel_spmd

F32 = mybir.dt.float32
BF16 = mybir.dt.bfloat16

B, H, W, C = 64, 32, 32, 512
A = 512
HID = 512
NCORES = 8
BL = B // NCORES          # batches per core
RPB = H * W               # rows (spatial positions) per batch
R = BL * RPB              # rows per core
P = 128
KT = C // P               # contraction tiles over C
AT = A // P               # tiles over attention dim
SUB = RPB // P            # 128-row subtiles per batch
HALF = 512                # psum free-dim half (PSUM bank = 512 f32)

Tanh = mybir.ActivationFunctionType.Tanh
Exp = mybir.ActivationFunctionType.Exp


def build():
    nc = bacc.Bacc(None, target_bir_lowering=False)

    feat = nc.declare_dram_parameter("features", [R, C], F32, isOutput=False)
    hid = nc.declare_dram_parameter("hidden", [BL, HID], F32, isOutput=False)
    maskp = nc.declare_dram_parameter("mask", [BL, RPB], F32, isOutput=False)
    w1p = nc.declare_dram_parameter("W1", [C, A], F32, isOutput=False)
    w2p = nc.declare_dram_parameter("W2", [HID, A], F32, isOutput=False)
    b2p = nc.declare_dram_parameter("b2", [A], F32, isOutput=False)
    vp = nc.declare_dram_parameter("V", [A, 1], F32, isOutput=False)
    bvp = nc.declare_dram_parameter("bV", [BL, 1], F32, isOutput=False)
    idp = nc.declare_dram_parameter("ident8", [BL, BL], F32, isOutput=False)
    octx = nc.declare_dram_parameter("out_ctx", [BL, C], F32, isOutput=True)
    oalpha = nc.declare_dram_parameter("out_alpha", [BL, RPB], F32, isOutput=True)

    with tile.TileContext(nc) as tc:
        with (
            tc.tile_pool(name="const", bufs=1) as cp,
            tc.tile_pool(name="xnat", bufs=1) as xp,
            tc.tile_pool(name="stream", bufs=2) as sp,
            tc.tile_pool(name="pf", bufs=2, space=bass.MemorySpace.PSUM) as pfp,
            tc.tile_pool(name="pp", bufs=1, space=bass.MemorySpace.PSUM) as ppp,
        ):
            # ---------------- constants / small setup ----------------
            vf = cp.tile([P, AT, 1], F32)
            nc.sync.dma_start(vf[:], vp[:].rearrange("(kt p) one -> p kt one", p=P))
            vb = cp.tile([P, AT, 1], BF16)
            nc.vector.tensor_copy(vb[:], vf[:])

            # batch-selector V: vsel[p, a, k, m] = V[a*128+p] if m == k else 0
            vsel = cp.tile([P, AT, BL, BL], BF16)
            nc.vector.memset(vsel[:], 0.0)
            for a in range(AT):
                for k in range(BL):
                    nc.vector.tensor_copy(
                        vsel[:, a, k, k : k + 1], vb[:, a, :]
                    )

            w1f = cp.tile([P, KT, A], F32)
            nc.sync.dma_start(w1f[:], w1p[:].rearrange("(kt p) a -> p kt a", p=P))
            w1b = cp.tile([P, KT, A], BF16)
            nc.vector.tensor_copy(w1b[:], w1f[:])

            w2f = cp.tile([P, KT, A], F32)
            nc.sync.dma_start(w2f[:], w2p[:].rearrange("(kt p) a -> p kt a", p=P))
            w2b = cp.tile([P, KT, A], BF16)
            nc.vector.tensor_copy(w2b[:], w2f[:])

            # b2 (+b1, folded host-side) as a K=1 matmul row
            b2row = cp.tile([1, A], F32)
            nc.sync.dma_start(b2row[:], b2p[:].rearrange("(one a) -> one a", one=1))
            b2rb = cp.tile([1, A], BF16)
            nc.vector.tensor_copy(b2rb[:], b2row[:])
            ones8 = cp.tile([1, BL], BF16)
            nc.vector.memset(ones8[:], 1.0)

            bvb = cp.tile([BL, 1], F32)
            nc.sync.dma_start(bvb[:], bvp[:])

            hidf = cp.tile([BL, HID], F32)
            nc.sync.dma_start(hidf[:], hid[:])
            hidb = cp.tile([BL, HID], BF16)
            nc.vector.tensor_copy(hidb[:], hidf[:])

            maskf = cp.tile([BL, RPB], F32)
            nc.sync.dma_start(maskf[:], maskp[:])
            maskc = cp.tile([BL, RPB], F32)
            nc.vector.tensor_copy(maskc[:], maskf[:])

            # identity (bf16) for PE transposes of [BL, 128] slices
            idf = cp.tile([BL, BL], F32)
            nc.sync.dma_start(idf[:], idp[:])
            ident = cp.tile([BL, BL], BF16)
            nc.vector.tensor_copy(ident[:], idf[:])

            # hiddenT: [HID, BL] as [P, KT, BL] via PE transpose
            p_hT = ppp.tile([P, KT, BL], BF16)
            for kt in range(KT):
                nc.tensor.transpose(p_hT[:, kt, :], hidb[0:BL, ts(kt, P)], ident[:])
            hidTb = cp.tile([P, KT, BL], BF16)
            nc.vector.tensor_copy(hidTb[:], p_hT[:])

            # h_projT[a_part, a_tile, b] = (hidden @ W2 + b2 + b1)^T
            # bias folded in as an extra K=1 contraction row (b2row x ones)
            p_h = ppp.tile([P, AT, BL], F32)
            for a in range(AT):
                for kk in range(KT):
                    nc.tensor.matmul(
                        p_h[:, a, :],
                        w2b[:, kk, ts(a, P)],
                        hidTb[:, kk, :],
                        start=(kk == 0),
                        stop=False,
                    )
                nc.tensor.matmul(
                    p_h[:, a, :],
                    b2rb[0:1, ts(a, P)],
                    ones8[:],
                    start=False,
                    stop=True,
                )
            hprojT = cp.tile([P, AT, BL], F32)
            nc.vector.tensor_copy(hprojT[:], p_h[:])

            # ACT observers: make the scalar engine observe the DVE tick of
            # hprojT and the DMA lane of bvb, so later activation bias reads
            # need no extra wait slot (ISA allows one sync wait per inst).
            obs_a = cp.tile([P, 1], F32)
            nc.scalar.copy(obs_a[:], hprojT[:, 0, 0:1])
            obs_b = cp.tile([BL, 1], F32)
            nc.scalar.copy(obs_b[:], bvb[:])

            # persistent bf16 features, natural layout [p, batch, sub, c]
            x16 = xp.tile([P, BL, SUB, C], BF16)

            # score psum accumulators [BL, 512] per half, one group across chunks
            p_sc0 = ppp.tile([BL, HALF], F32, tag="p_sc0")
            p_sc1 = ppp.tile([BL, HALF], F32, tag="p_sc1")
            p_sc = [p_sc0, p_sc1]

            # ---------------- main loop over batches (chunks of 1024 rows) ----
            for k in range(BL):
                xraw = sp.tile([P, SUB, C], F32, tag="xraw")
                nc.sync.dma_start(
                    xraw[:],
                    feat[k * RPB : (k + 1) * RPB, :].rearrange(
                        "(s p) c -> p s c", p=P
                    ),
                )
                nc.vector.tensor_copy(x16[:, k, :, :], xraw[:])

                # X^T via one X-bar DMA transpose of the whole chunk:
                # xT[c % 128, s, c // 128, row % 128]
                xT = sp.tile([P, SUB, KT, P], BF16, tag="xT")
                nc.scalar.dma_start(xT[:], x16[:, k, :, :], transpose=True)

                tanh_t = sp.tile([P, AT, RPB], BF16, tag="tanh")
                for a in range(AT):
                    for half in range(2):
                        pf = pfp.tile([P, HALF], F32, tag="pf")
                        for kc in range(KT):
                            nc.tensor.matmul(
                                pf[:],
                                w1b[:, kc, ts(a, P)],
                                xT[:, ds(half * 4, 4), kc, :],
                                start=(kc == 0),
                                stop=(kc == KT - 1),
                            )
                        nc.scalar.activation(
                            tanh_t[:, a, ts(half, HALF)],
                            pf[:],
                            Tanh,
                            bias=hprojT[:, a, k : k + 1],
                        )

                # V-dot into batch-row k of the score accumulators
                for half in range(2):
                    for a in range(AT):
                        nc.tensor.matmul(
                            p_sc[half][:],
                            vsel[:, a, k, :],
                            tanh_t[:, a, ts(half, HALF)],
                            start=(k == 0 and a == 0),
                            stop=(k == BL - 1 and a == AT - 1),
                        )

            # ---------------- softmax (no max-sub; scores are O(1)) ----------
            w_sb = cp.tile([BL, RPB], F32)
            for half in range(2):
                nc.scalar.activation(
                    w_sb[:, ts(half, HALF)], p_sc[half][:], Exp, bias=bvb[:]
                )
            wm = cp.tile([BL, RPB], F32)
            nc.vector.tensor_tensor(
                wm[:], w_sb[:], maskc[:], mybir.AluOpType.mult
            )
            rsum = cp.tile([BL, 1], F32)
            nc.vector.tensor_reduce(
                rsum[:], wm[:], mybir.AxisListType.X, mybir.AluOpType.add
            )
            rinv = cp.tile([BL, 1], F32)
            nc.vector.reciprocal(rinv[:], rsum[:])

            alpha_sb = cp.tile([BL, RPB], F32)
            nc.vector.tensor_scalar_mul(alpha_sb[:], wm[:], rinv[:])
            nc.sync.dma_start(oalpha[:], alpha_sb[:])

            # ---------------- context = (w @ X) / sum ------------------------
            wmb = cp.tile([BL, RPB], BF16)
            nc.vector.tensor_copy(wmb[:], wm[:])

            p_bdT = ppp.tile([P, SUB, BL], BF16)
            for s in range(SUB):
                nc.tensor.transpose(p_bdT[:, s, :], wmb[0:BL, ts(s, P)], ident[:])

            bd = cp.tile([P, BL * SUB, BL], BF16)
            nc.vector.memset(bd[:], 0.0)
            for b in range(BL):
                nc.vector.tensor_copy(
                    bd[:, ds(b * SUB, SUB), b : b + 1], p_bdT[:, :, b : b + 1]
                )

            p_ctx = ppp.tile([BL, C], F32)
            for b in range(BL):
                for s in range(SUB):
                    nc.tensor.matmul(
                        p_ctx[:],
                        bd[:, b * SUB + s, :],
                        x16[:, b, s, :],
                        start=(b == 0 and s == 0),
                        stop=(b == BL - 1 and s == SUB - 1),
                    )
            ctxf = cp.tile([BL, C], F32)
            nc.vector.tensor_copy(ctxf[:], p_ctx[:])
            ctx_sb = cp.tile([BL, C], F32)
            nc.vector.tensor_scalar_mul(ctx_sb[:], ctxf[:], rinv[:])
            nc.sync.dma_start(octx[:], ctx_sb[:])

    nc.compile()
    return nc


_NC = None


def _get_nc():
    global _NC
    if _NC is None:
        _NC = build()
    return _NC


def run(inputs, trace=False, trace_kwargs=None):
    feats = np.ascontiguousarray(inputs["features"], dtype=np.float32)
    hidden = np.ascontiguousarray(inputs["hidden"], dtype=np.float32)
    mask = np.ascontiguousarray(inputs["mask"]).astype(np.float32)
    w1 = np.ascontiguousarray(inputs["W1"], dtype=np.float32)
    b1 = np.ascontiguousarray(inputs["b1"], dtype=np.float32)
    w2 = np.ascontiguousarray(inputs["W2"], dtype=np.float32)
    b2 = np.ascontiguousarray(inputs["b2"], dtype=np.float32)
    v = np.ascontiguousarray(inputs["V"], dtype=np.float32)
    bv = np.ascontiguousarray(inputs["bV"], dtype=np.float32)

    in_maps = []
    for i in range(NCORES):
        sl = slice(i * BL, (i + 1) * BL)
        in_maps.append(
            {
                "features": feats[sl].reshape(R, C),
                "hidden": hidden[sl],
                "mask": mask[sl].reshape(BL, RPB),
                "W1": w1,
                "W2": w2,
                "b2": b2 + b1,
                "V": v,
                "bV": np.full((BL, 1), float(bv.reshape(-1)[0]), dtype=np.float32),
                "ident8": np.eye(BL, dtype=np.float32),
            }
        )

    nc = _get_nc()
    kw = {}
    if trace:
        kw["trace"] = True
        if trace_kwargs:
            kw["trace_kwargs"] = trace_kwargs
    res = run_bass_kernel_spmd(nc, in_maps, list(range(NCORES)), **kw)

    ctx = np.concatenate([r["out_ctx"] for r in res.results], axis=0)
    alpha = np.concatenate([r["out_alpha"] for r in res.results], axis=0)
    alpha = alpha.reshape(B, H, W)
    return (ctx.astype(np.float32), alpha.astype(np.float32)), res


def kernel(**inputs):
    (ctx, alpha), _ = run(inputs, trace=False)
    return ctx, alpha
